# revision 1
# baseline (speedup 1.0000x reference)
"""Trainium2 Bass kernel for a dense decoder block (pre-LN MHA + FFN).

Shapes (hardcoded): B=512, T=128, D=384, H=6, DH=64, DFF=1536.
Sharding: pure data parallel -- batch split 64-per-core across 8 cores,
all weights replicated.

Per-core kernel layout:
  * Each sequence's T=128 tokens sit on the 128 SBUF partitions
    (token-major [T, D] tiles); LN stats are free-dim reductions.
  * Sequences are processed in groups of GRP=4 so the projection /
    FFN matmuls stream 512-wide moving operands (full PE efficiency).
  * Activations feeding the PE are cast to fp16 (1 cycle/row at any N);
    accumulation stays fp32 in PSUM, the residual stream stays fp32.
  * Host-side folding: LN gains g1/g2 are folded into Wq/Wk/Wv/W1;
    be1@Wv@Wo is folded into the attention-output bias; be2@W1 into the
    FFN1 bias.  Softmax runs unnormalized (exp with fp32 row-sum
    accumulated by the ACT engine) and the 1/l factor is applied to the
    attention probabilities before the attn@V matmul.
"""

import os
import sys
from contextlib import ExitStack

import numpy as np

for _p in ("/opt/trn_rl_repo", "/root/.axon_site/_ro/trn_rl_repo"):
    if os.path.isdir(_p) and _p not in sys.path:
        sys.path.append(_p)

import concourse.bass as bass
import concourse.tile as tile
from concourse import bacc, mybir
from concourse.masks import make_causal_mask, make_identity

B, T, D, H = 512, 128, 384, 6
DH = D // H          # 64
DFF = 4 * D          # 1536
EPS = 1e-5
N_CORES = 8
BL = B // N_CORES    # 64 sequences per core
GRP = 4              # sequences per compute group (512-wide moving dims)

F32 = mybir.dt.float32
F16 = mybir.dt.float16
DC = D // 128        # 3 chunks of the model dim
FC = DFF // 128      # 12 chunks of the FFN dim
SCALE = DH ** -0.5   # 0.125

ACT = mybir.ActivationFunctionType


def _copy(eng, out, in_):
    if eng is eng.bass.scalar:
        eng.copy(out=out, in_=in_)
    else:
        eng.tensor_copy(out=out, in_=in_)


def build_decoder_block(tc, io, bl, grp):
    nc = tc.nc
    ctx = ExitStack()
    with ctx:
        _build(ctx, tc, nc, io, bl, grp)


def _build(ctx, tc, nc, io, bl, grp):
    x_d = io["x"]
    out_d = io["out"]
    gt = grp * T

    singles = ctx.enter_context(tc.tile_pool(name="singles", bufs=1))

    ident = singles.tile([128, 128], F16)
    make_identity(nc, ident)
    mask = singles.tile([128, 128], F32)
    make_causal_mask(nc, mask, mask_val=-1e10)

    def load_w(name, nchunks, width):
        tiles = []
        for c in range(nchunks):
            t = singles.tile([128, width], F16, tag=f"{name}_{c}",
                             name=f"{name}_{c}")
            nc.sync.dma_start(out=t, in_=io[name][c * 128:(c + 1) * 128, :])
            tiles.append(t)
        return tiles

    wq = load_w("wq", DC, D)
    wk = load_w("wk", DC, D)
    wv = load_w("wv", DC, D)
    wo = load_w("wo", DC, D)
    w1 = load_w("w1", DC, DFF)
    w2 = load_w("w2", FC, D)

    bq = singles.tile([128, DC], F32)
    nc.sync.dma_start(out=bq, in_=io["bq"].rearrange("(c p) -> p c", p=128))
    bk = singles.tile([128, DC], F32)
    nc.sync.dma_start(out=bk, in_=io["bk"].rearrange("(c p) -> p c", p=128))
    b1 = singles.tile([128, FC], F32)
    nc.sync.dma_start(out=b1, in_=io["b1p"].rearrange("(c p) -> p c", p=128))

    def bcast_row(name):
        t = singles.tile([128, D], F32, tag=f"bc_{name}", name=f"bc_{name}")
        src = io[name]
        nc.sync.dma_start(
            out=t,
            in_=bass.AP(tensor=src.tensor, offset=src.offset,
                        ap=[[0, 128]] + list(src.ap)),
        )
        return t

    bo2 = bcast_row("bo2")
    b2 = bcast_row("b2")

    eps_t = singles.tile([128, 1], F32)
    nc.vector.memset(eps_t, EPS)

    psum = ctx.enter_context(tc.tile_pool(name="psum", bufs=8, space="PSUM"))
    xp = ctx.enter_context(tc.tile_pool(name="xp", bufs=grp + 2))
    stp = ctx.enter_context(tc.tile_pool(name="stp", bufs=4 * grp))
    lnp = ctx.enter_context(tc.tile_pool(name="lnp", bufs=3))
    htp = ctx.enter_context(tc.tile_pool(name="htp", bufs=2))
    qkp = ctx.enter_context(tc.tile_pool(name="qkp", bufs=2))
    vp = ctx.enter_context(tc.tile_pool(name="vp", bufs=grp + 2))
    pp = ctx.enter_context(tc.tile_pool(name="pp", bufs=4))
    pnp = ctx.enter_context(tc.tile_pool(name="pnp", bufs=2 * H * grp // 2 + 2))
    otp = ctx.enter_context(tc.tile_pool(name="otp", bufs=2))
    o1p = ctx.enter_context(tc.tile_pool(name="o1p", bufs=2 * grp + 2))
    ap_ = ctx.enter_context(tc.tile_pool(name="ap", bufs=2))
    obp = ctx.enter_context(tc.tile_pool(name="obp", bufs=3))

    def layer_norm_f16(src_tile):
        st = stp.tile([128, 6], F32, tag="st")
        nc.vector.bn_stats(out=st, in_=src_tile)
        mv = stp.tile([128, 2], F32, tag="mv")
        nc.vector.bn_aggr(out=mv, in_=st)
        sd = stp.tile([128, 1], F32, tag="sd")
        nc.scalar.activation(out=sd, in_=mv[:, 1:2], func=ACT.Sqrt,
                             bias=eps_t, scale=1.0)
        rs = stp.tile([128, 1], F32, tag="rs")
        nc.vector.reciprocal(out=rs, in_=sd)
        ln = lnp.tile([128, D], F16, tag="ln", name="ln")
        nc.vector.tensor_scalar(
            out=ln, in0=src_tile, scalar1=mv[:, 0:1], scalar2=rs,
            op0=mybir.AluOpType.subtract, op1=mybir.AluOpType.mult)
        return ln

    def transpose_into(dst, dst_col, ln):
        for c in range(DC):
            pt = psum.tile([128, 128], F16, tag="ps", name="pt")
            nc.tensor.transpose(pt, ln[:, c * 128:(c + 1) * 128], ident)
            _copy(nc.vector if c % 2 else nc.scalar,
                  dst[c][:, dst_col:dst_col + T], pt)

    n_groups = bl // grp
    prev_d = []
    for g in range(n_groups):
        d_iter = iter(prev_d)

        def emit_d(n=1):
            for _ in range(n):
                for d in d_iter:
                    d()
                    break

        xb = []
        hT = [htp.tile([128, gt], F16, tag=f"hT_{c}", name=f"hT_{c}")
              for c in range(DC)]
        for j in range(grp):
            b = g * grp + j
            xt = xp.tile([128, D], F32, tag="x", name="xt")
            nc.sync.dma_start(out=xt, in_=x_d[b])
            xb.append(xt)
            ln1 = layer_norm_f16(xt)
            transpose_into(hT, j * T, ln1)
            emit_d(1)

        qT, kT = [], []
        for mc in range(DC):
            pq = psum.tile([128, gt], F32, tag="ps", name="pq")
            for kc in range(DC):
                nc.tensor.matmul(pq, lhsT=wq[kc][:, mc * 128:(mc + 1) * 128],
                                 rhs=hT[kc], start=(kc == 0),
                                 stop=(kc == DC - 1))
            qs = qkp.tile([128, gt], F16, tag=f"q_{mc}", name=f"q_{mc}")
            nc.vector.tensor_scalar(
                out=qs, in0=pq, scalar1=bq[:, mc:mc + 1], scalar2=SCALE,
                op0=mybir.AluOpType.add, op1=mybir.AluOpType.mult)
            qT.append(qs)

            pk = psum.tile([128, gt], F32, tag="ps", name="pk")
            for kc in range(DC):
                nc.tensor.matmul(pk, lhsT=wk[kc][:, mc * 128:(mc + 1) * 128],
                                 rhs=hT[kc], start=(kc == 0),
                                 stop=(kc == DC - 1))
            ks = qkp.tile([128, gt], F16, tag=f"k_{mc}", name=f"k_{mc}")
            nc.vector.tensor_scalar_add(out=ks, in0=pk,
                                        scalar1=bk[:, mc:mc + 1])
            kT.append(ks)
            emit_d(1)

        vs = []
        for j in range(grp):
            pv = psum.tile([128, D], F32, tag="ps", name="pv")
            for kc in range(DC):
                nc.tensor.matmul(pv, lhsT=hT[kc][:, j * T:(j + 1) * T],
                                 rhs=wv[kc], start=(kc == 0),
                                 stop=(kc == DC - 1))
            vt = vp.tile([128, D], F16, tag="v", name="vt")
            nc.scalar.copy(out=vt, in_=pv)
            vs.append(vt)

        # ---- attention in two same-stage waves (v1 ops, new order) ----
        # Phase A: per (j,h) softmax chain; scores PSUM freed at exp so
        # each engine runs a dense stream of one op kind.  Phase B:
        # transpose + copy + attn@V, j-grouped.  FFN thunks of group g-1
        # are sprinkled between blocks to keep the PE busy.
        oT = [otp.tile([128, gt], F16, tag=f"oT_{c}", name=f"oT_{c}")
              for c in range(DC)]
        pns = {}
        for j in range(grp):
            for h in range(H):
                mc, off = h // 2, (h % 2) * 64
                jj = slice(j * T, (j + 1) * T)
                ps = psum.tile([128, T], F32, tag="ps", name="ps")
                nc.tensor.matmul(ps, lhsT=qT[mc][off:off + 64, jj],
                                 rhs=kT[mc][off:off + 64, jj])
                nc.vector.tensor_add(out=ps, in0=ps, in1=mask)
                p16 = pp.tile([128, T], F16, tag="p", name="p16")
                lsum = stp.tile([128, 1], F32, tag="l")
                nc.scalar.activation(out=p16, in_=ps, func=ACT.Exp,
                                     accum_out=lsum)
                rl = stp.tile([128, 1], F32, tag="rl")
                nc.vector.reciprocal(out=rl, in_=lsum)
                pn = pnp.tile([128, T], F16, tag="pn", name="pn")
                nc.vector.tensor_scalar_mul(out=pn, in0=p16, scalar1=rl)
                pns[j, h] = pn
                if h % 2 == 1:
                    emit_d(1)
        for j in range(grp):
            po = None
            jj = slice(j * T, (j + 1) * T)
            for h in range(H):
                mc, off = h // 2, (h % 2) * 64
                ptp = psum.tile([128, T], F16, tag="ps", name="ptp")
                nc.tensor.transpose(ptp, pns[j, h], ident)
                pT = pp.tile([128, T], F16, tag="pT", name="pT")
                _copy(nc.vector if h % 2 else nc.scalar, pT, ptp)
                if h % 2 == 0:
                    po = psum.tile([128, T], F32, tag="ps", name="po")
                nc.tensor.matmul(po[off:off + 64, :],
                                 lhsT=vs[j][:, h * DH:(h + 1) * DH], rhs=pT)
                if h % 2 == 1:
                    _copy(nc.vector if j % 2 else nc.scalar,
                          oT[mc][:, jj], po)
            emit_d(1)

        o1s = []
        h2T = [htp.tile([128, gt], F16, tag=f"h2T_{c}", name=f"h2T_{c}")
               for c in range(DC)]
        for j in range(grp):
            jj = slice(j * T, (j + 1) * T)
            pr = psum.tile([128, D], F32, tag="ps", name="pr")
            for kc in range(DC):
                nc.tensor.matmul(pr, lhsT=oT[kc][:, jj], rhs=wo[kc],
                                 start=(kc == 0), stop=(kc == DC - 1))
            o1 = o1p.tile([128, D], F32, tag="o1", name="o1")
            nc.vector.tensor_add(out=o1, in0=pr, in1=xb[j])
            nc.gpsimd.tensor_add(out=o1, in0=o1, in1=bo2)
            o1s.append(o1)
            ln2 = layer_norm_f16(o1)
            transpose_into(h2T, j * T, ln2)
            emit_d(1)
        emit_d(len(prev_d))

        # ---- FFN of this group, deferred into the next group ----
        def make_d(g, h2T, o1s):
            a1 = []
            ops = []

            def a1_chunk(mf):
                def run():
                    pa = psum.tile([128, gt], F32, tag="ps", name="pa")
                    for kc in range(DC):
                        nc.tensor.matmul(
                            pa, lhsT=w1[kc][:, mf * 128:(mf + 1) * 128],
                            rhs=h2T[kc], start=(kc == 0),
                            stop=(kc == DC - 1))
                    at = ap_.tile([128, gt], F16, tag=f"a_{mf}",
                                  name=f"a_{mf}")
                    nc.scalar.activation(out=at, in_=pa, func=ACT.Relu,
                                         bias=b1[:, mf:mf + 1], scale=1.0)
                    a1.append(at)
                return run

            def ff_j(j):
                def run():
                    jj = slice(j * T, (j + 1) * T)
                    pf = psum.tile([128, D], F32, tag="ps", name="pf")
                    for kc in range(FC):
                        nc.tensor.matmul(pf, lhsT=a1[kc][:, jj],
                                         rhs=w2[kc], start=(kc == 0),
                                         stop=(kc == FC - 1))
                    ob = obp.tile([128, D], F32, tag="ob", name="ob")
                    nc.vector.tensor_add(out=ob, in0=pf, in1=o1s[j])
                    nc.gpsimd.tensor_add(out=ob, in0=ob, in1=b2)
                    nc.sync.dma_start(out=out_d[g * grp + j], in_=ob)
                return run

            for mf in range(FC):
                ops.append(a1_chunk(mf))
            for j in range(grp):
                ops.append(ff_j(j))
            return ops

        prev_d = make_d(g, h2T, o1s)

    for d in prev_d:
        d()


def build_nc(bl=BL, grp=GRP):
    nc = bacc.Bacc("TRN2", target_bir_lowering=False, debug=False,
                   enable_asserts=True)
    io = {}

    def inp(name, shape, dt):
        io[name] = nc.dram_tensor(name, shape, dt, kind="ExternalInput").ap()

    inp("x", [bl, T, D], F32)
    inp("wq", [D, D], F16)
    inp("wk", [D, D], F16)
    inp("wv", [D, D], F16)
    inp("wo", [D, D], F16)
    inp("w1", [D, DFF], F16)
    inp("w2", [DFF, D], F16)
    inp("bq", [D], F32)
    inp("bk", [D], F32)
    inp("b1p", [DFF], F32)
    inp("bo2", [D], F32)
    inp("b2", [D], F32)
    io["out"] = nc.dram_tensor("out", [bl, T, D], F32,
                               kind="ExternalOutput").ap()

    with tile.TileContext(nc) as tc:
        build_decoder_block(tc, io, bl, grp)
    nc.compile()
    return nc


def prep_weights(Wq, Wk, Wv, Wo, bo, W1, b1, W2, b2, g1, be1, g2, be2):
    f = np.float64
    Wq, Wk, Wv, Wo = (np.asarray(a, f) for a in (Wq, Wk, Wv, Wo))
    W1, W2 = np.asarray(W1, f), np.asarray(W2, f)
    g1, be1, g2, be2 = (np.asarray(a, f) for a in (g1, be1, g2, be2))
    bo, b1, b2 = np.asarray(bo, f), np.asarray(b1, f), np.asarray(b2, f)
    return {
        "wq": (g1[:, None] * Wq).astype(np.float16),
        "wk": (g1[:, None] * Wk).astype(np.float16),
        "wv": (g1[:, None] * Wv).astype(np.float16),
        "wo": Wo.astype(np.float16),
        "w1": (g2[:, None] * W1).astype(np.float16),
        "w2": W2.astype(np.float16),
        "bq": (be1 @ Wq).astype(np.float32),
        "bk": (be1 @ Wk).astype(np.float32),
        "b1p": (b1 + be2 @ W1).astype(np.float32),
        "bo2": (bo + (be1 @ Wv) @ Wo).astype(np.float32),
        "b2": b2.astype(np.float32),
    }


_NC_CACHE = {}


def get_nc(bl=BL, grp=GRP):
    key = (bl, grp)
    if key not in _NC_CACHE:
        _NC_CACHE[key] = build_nc(bl, grp)
    return _NC_CACHE[key]


def kernel(**inputs):
    from concourse.bass_utils import run_bass_kernel_spmd

    x = np.asarray(inputs["x"], np.float32)
    w = prep_weights(**{k: v for k, v in inputs.items() if k != "x"})
    nc = get_nc()
    in_maps = []
    for c in range(N_CORES):
        m = dict(w)
        m["x"] = np.ascontiguousarray(x[c * BL:(c + 1) * BL])
        in_maps.append(m)
    res = run_bass_kernel_spmd(nc, in_maps, list(range(N_CORES)))
    return np.concatenate([r["out"] for r in res.results], axis=0)



# revision 9
# speedup vs baseline: 79.0050x; 79.0050x over previous
"""Trainium2 Bass kernel for a dense decoder block (pre-LN MHA + FFN).

Shapes (hardcoded): B=512, T=128, D=384, H=6, DH=64, DFF=1536.
Sharding: pure data parallel -- batch split 64-per-core across 8 cores,
all weights replicated.

Per-core kernel layout:
  * Each sequence's T=128 tokens sit on the 128 SBUF partitions
    (token-major [T, D] tiles); LN stats are free-dim reductions.
  * Sequences are processed in groups of GRP=4 so the projection /
    FFN matmuls stream 512-wide moving operands (full PE efficiency).
  * Activations feeding the PE are cast to fp16 (1 cycle/row at any N);
    accumulation stays fp32 in PSUM, the residual stream stays fp32.
  * Host-side folding: LN gains g1/g2 are folded into Wq/Wk/Wv/W1;
    be1@Wv@Wo is folded into the attention-output bias; be2@W1 into the
    FFN1 bias.  Softmax runs unnormalized (exp with fp32 row-sum
    accumulated by the ACT engine) and the 1/l factor is applied to the
    attention probabilities before the attn@V matmul.
"""

import os
import sys
from contextlib import ExitStack

import numpy as np

for _p in ("/opt/trn_rl_repo", "/root/.axon_site/_ro/trn_rl_repo"):
    if os.path.isdir(_p) and _p not in sys.path:
        sys.path.append(_p)

import concourse.bass as bass
import concourse.tile as tile
from concourse import bacc, mybir
from concourse.masks import make_causal_mask, make_identity

B, T, D, H = 512, 128, 384, 6
DH = D // H          # 64
DFF = 4 * D          # 1536
EPS = 1e-5
N_CORES = 8
BL = B // N_CORES    # 64 sequences per core
GRP = 4              # sequences per compute group (512-wide moving dims)

F32 = mybir.dt.float32
F16 = mybir.dt.float16
DC = D // 128        # 3 chunks of the model dim
FC = DFF // 128      # 12 chunks of the FFN dim
SCALE = DH ** -0.5   # 0.125

ACT = mybir.ActivationFunctionType


def _copy(eng, out, in_):
    if eng is eng.bass.scalar:
        eng.copy(out=out, in_=in_)
    else:
        eng.tensor_copy(out=out, in_=in_)


def build_decoder_block(tc, io, bl, grp, use_bias):
    nc = tc.nc
    ctx = ExitStack()
    with ctx:
        _build(ctx, tc, nc, io, bl, grp, use_bias)


def _build(ctx, tc, nc, io, bl, grp, use_bias):
    x_d = io["x"]
    out_d = io["out"]
    gt = grp * T

    singles = ctx.enter_context(tc.tile_pool(name="singles", bufs=1))

    ident = singles.tile([128, 128], F16)
    make_identity(nc, ident)
    mask = singles.tile([128, 128], F32)
    make_causal_mask(nc, mask, mask_val=-1e10)

    def load_w(name, nchunks, width):
        tiles = []
        for c in range(nchunks):
            t = singles.tile([128, width], F16, tag=f"{name}_{c}",
                             name=f"{name}_{c}")
            nc.sync.dma_start(out=t, in_=io[name][c * 128:(c + 1) * 128, :])
            tiles.append(t)
        return tiles

    wq = load_w("wq", DC, D)
    wk = load_w("wk", DC, D)
    wv = load_w("wv", DC, D)
    wo = load_w("wo", DC, D)
    w1 = load_w("w1", DC, DFF)
    w2 = load_w("w2", FC, D)

    if use_bias:
        bq = singles.tile([128, DC], F32)
        nc.sync.dma_start(out=bq,
                          in_=io["bq"].rearrange("(c p) -> p c", p=128))
        bk = singles.tile([128, DC], F32)
        nc.sync.dma_start(out=bk,
                          in_=io["bk"].rearrange("(c p) -> p c", p=128))
        b1 = singles.tile([128, FC], F32)
        nc.sync.dma_start(out=b1,
                          in_=io["b1p"].rearrange("(c p) -> p c", p=128))

        def bcast_row(name):
            t = singles.tile([128, D], F32, tag=f"bc_{name}", name=f"bc_{name}")
            src = io[name]
            nc.sync.dma_start(
                out=t,
                in_=bass.AP(tensor=src.tensor, offset=src.offset,
                            ap=[[0, 128]] + list(src.ap)),
            )
            return t

        bo2 = bcast_row("bo2")
        b2 = bcast_row("b2")

    eps_t = singles.tile([128, 1], F32)
    nc.vector.memset(eps_t, EPS)

    psum = ctx.enter_context(tc.tile_pool(name="psum", bufs=8, space="PSUM"))
    xp = ctx.enter_context(tc.tile_pool(name="xp", bufs=grp + 2))
    stp = ctx.enter_context(tc.tile_pool(name="stp", bufs=4 * grp))
    lnp = ctx.enter_context(tc.tile_pool(name="lnp", bufs=3))
    htp = ctx.enter_context(tc.tile_pool(name="htp", bufs=2))
    qkp = ctx.enter_context(tc.tile_pool(name="qkp", bufs=2))
    vp = ctx.enter_context(tc.tile_pool(name="vp", bufs=grp + 2))
    pp = ctx.enter_context(tc.tile_pool(name="pp", bufs=4))
    pnp = ctx.enter_context(tc.tile_pool(name="pnp", bufs=2 * H * grp // 2 + 2))
    otp = ctx.enter_context(tc.tile_pool(name="otp", bufs=2))
    o1p = ctx.enter_context(tc.tile_pool(name="o1p", bufs=2 * grp + 2))
    ap_ = ctx.enter_context(tc.tile_pool(name="ap", bufs=2))
    obp = ctx.enter_context(tc.tile_pool(name="obp", bufs=3))

    def layer_norm_f16(src_tile):
        st = stp.tile([128, 6], F32, tag="st")
        nc.vector.bn_stats(out=st, in_=src_tile)
        mv = stp.tile([128, 2], F32, tag="mv")
        nc.vector.bn_aggr(out=mv, in_=st)
        sd = stp.tile([128, 1], F32, tag="sd")
        nc.scalar.activation(out=sd, in_=mv[:, 1:2], func=ACT.Sqrt,
                             bias=eps_t, scale=1.0)
        rs = stp.tile([128, 1], F32, tag="rs")
        nc.vector.reciprocal(out=rs, in_=sd)
        ln = lnp.tile([128, D], F16, tag="ln", name="ln")
        nc.vector.tensor_scalar(
            out=ln, in0=src_tile, scalar1=mv[:, 0:1], scalar2=rs,
            op0=mybir.AluOpType.subtract, op1=mybir.AluOpType.mult)
        return ln

    def transpose_into(dst, dst_col, ln):
        for c in range(DC):
            pt = psum.tile([128, 128], F16, tag="ps", name="pt")
            nc.tensor.transpose(pt, ln[:, c * 128:(c + 1) * 128], ident)
            _copy(nc.vector if c % 2 else nc.scalar,
                  dst[c][:, dst_col:dst_col + T], pt)

    n_groups = bl // grp
    prev_d = []
    for g in range(n_groups):
        d_iter = iter(prev_d)

        def emit_d(n=1):
            for _ in range(n):
                for d in d_iter:
                    d()
                    break

        xb = []
        hT = [htp.tile([128, gt], F16, tag=f"hT_{c}", name=f"hT_{c}")
              for c in range(DC)]
        for j in range(grp):
            b = g * grp + j
            xt = xp.tile([128, D], F32, tag="x", name="xt")
            nc.sync.dma_start(out=xt, in_=x_d[b])
            xb.append(xt)
            ln1 = layer_norm_f16(xt)
            transpose_into(hT, j * T, ln1)
            emit_d(1)

        qT, kT = [], []
        for mc in range(DC):
            pq = psum.tile([128, gt], F32, tag="ps", name="pq")
            for kc in range(DC):
                nc.tensor.matmul(pq, lhsT=wq[kc][:, mc * 128:(mc + 1) * 128],
                                 rhs=hT[kc], start=(kc == 0),
                                 stop=(kc == DC - 1))
            qs = qkp.tile([128, gt], F16, tag=f"q_{mc}", name=f"q_{mc}")
            if use_bias:
                nc.vector.tensor_scalar_add(out=qs, in0=pq,
                                            scalar1=bq[:, mc:mc + 1])
            else:
                nc.scalar.copy(out=qs, in_=pq)
            qT.append(qs)

            pk = psum.tile([128, gt], F32, tag="ps", name="pk")
            for kc in range(DC):
                nc.tensor.matmul(pk, lhsT=wk[kc][:, mc * 128:(mc + 1) * 128],
                                 rhs=hT[kc], start=(kc == 0),
                                 stop=(kc == DC - 1))
            ks = qkp.tile([128, gt], F16, tag=f"k_{mc}", name=f"k_{mc}")
            if use_bias:
                nc.vector.tensor_scalar_add(out=ks, in0=pk,
                                            scalar1=bk[:, mc:mc + 1])
            else:
                nc.vector.tensor_copy(out=ks, in_=pk)
            kT.append(ks)
            emit_d(1)

        vs = []
        for j in range(grp):
            pv = psum.tile([128, D], F32, tag="ps", name="pv")
            for kc in range(DC):
                nc.tensor.matmul(pv, lhsT=hT[kc][:, j * T:(j + 1) * T],
                                 rhs=wv[kc], start=(kc == 0),
                                 stop=(kc == DC - 1))
            vt = vp.tile([128, D], F16, tag="v", name="vt")
            nc.scalar.copy(out=vt, in_=pv)
            vs.append(vt)

        # ---- attention in two same-stage waves (v1 ops, new order) ----
        # Phase A: per (j,h) softmax chain; scores PSUM freed at exp so
        # each engine runs a dense stream of one op kind.  Phase B:
        # transpose + copy + attn@V, j-grouped.  FFN thunks of group g-1
        # are sprinkled between blocks to keep the PE busy.
        oT = [otp.tile([128, gt], F16, tag=f"oT_{c}", name=f"oT_{c}")
              for c in range(DC)]
        pns = {}
        for j in range(grp):
            for h in range(H):
                mc, off = h // 2, (h % 2) * 64
                jj = slice(j * T, (j + 1) * T)
                ps = psum.tile([128, T], F32, tag="ps", name="ps")
                nc.tensor.matmul(ps, lhsT=qT[mc][off:off + 64, jj],
                                 rhs=kT[mc][off:off + 64, jj])
                nc.vector.tensor_add(out=ps, in0=ps, in1=mask)
                p16 = pp.tile([128, T], F16, tag="p", name="p16")
                lsum = stp.tile([128, 1], F32, tag="l")
                nc.scalar.activation(out=p16, in_=ps, func=ACT.Exp,
                                     accum_out=lsum)
                rl = stp.tile([128, 1], F32, tag="rl")
                nc.vector.reciprocal(out=rl, in_=lsum)
                pn = pnp.tile([128, T], F16, tag="pn", name="pn")
                nc.vector.tensor_scalar_mul(out=pn, in0=p16, scalar1=rl)
                pns[j, h] = pn
                if h % 2 == 1:
                    emit_d(1)
        for j in range(grp):
            po = None
            jj = slice(j * T, (j + 1) * T)
            for h in range(H):
                mc, off = h // 2, (h % 2) * 64
                ptp = psum.tile([128, T], F16, tag="ps", name="ptp")
                nc.tensor.transpose(ptp, pns[j, h], ident)
                pT = pp.tile([128, T], F16, tag="pT", name="pT")
                _copy(nc.vector if h % 2 else nc.scalar, pT, ptp)
                if h % 2 == 0:
                    po = psum.tile([128, T], F32, tag="ps", name="po")
                nc.tensor.matmul(po[off:off + 64, :],
                                 lhsT=vs[j][:, h * DH:(h + 1) * DH], rhs=pT)
                if h % 2 == 1:
                    _copy(nc.vector if j % 2 else nc.scalar,
                          oT[mc][:, jj], po)
            emit_d(1)

        o1s = []
        h2T = [htp.tile([128, gt], F16, tag=f"h2T_{c}", name=f"h2T_{c}")
               for c in range(DC)]
        for j in range(grp):
            jj = slice(j * T, (j + 1) * T)
            pr = psum.tile([128, D], F32, tag="ps", name="pr")
            for kc in range(DC):
                nc.tensor.matmul(pr, lhsT=oT[kc][:, jj], rhs=wo[kc],
                                 start=(kc == 0), stop=(kc == DC - 1))
            o1 = o1p.tile([128, D], F32, tag="o1", name="o1")
            nc.vector.tensor_add(out=o1, in0=pr, in1=xb[j])
            if use_bias:
                nc.gpsimd.tensor_add(out=o1, in0=o1, in1=bo2)
            o1s.append(o1)
            ln2 = layer_norm_f16(o1)
            transpose_into(h2T, j * T, ln2)
            emit_d(1)
        emit_d(len(prev_d))

        # ---- FFN of this group, deferred into the next group ----
        def make_d(g, h2T, o1s):
            a1 = []
            ops = []

            def a1_chunk(mf):
                def run():
                    pa = psum.tile([128, gt], F32, tag="ps", name="pa")
                    for kc in range(DC):
                        nc.tensor.matmul(
                            pa, lhsT=w1[kc][:, mf * 128:(mf + 1) * 128],
                            rhs=h2T[kc], start=(kc == 0),
                            stop=(kc == DC - 1))
                    at = ap_.tile([128, gt], F16, tag=f"a_{mf}",
                                  name=f"a_{mf}")
                    nc.scalar.activation(out=at, in_=pa, func=ACT.Relu,
                                         bias=(b1[:, mf:mf + 1] if use_bias
                                               else 0.0), scale=1.0)
                    a1.append(at)
                return run

            def ff_j(j):
                def run():
                    jj = slice(j * T, (j + 1) * T)
                    pf = psum.tile([128, D], F32, tag="ps", name="pf")
                    for kc in range(FC):
                        nc.tensor.matmul(pf, lhsT=a1[kc][:, jj],
                                         rhs=w2[kc], start=(kc == 0),
                                         stop=(kc == FC - 1))
                    ob = obp.tile([128, D], F32, tag="ob", name="ob")
                    nc.vector.tensor_add(out=ob, in0=pf, in1=o1s[j])
                    if use_bias:
                        nc.gpsimd.tensor_add(out=ob, in0=ob, in1=b2)
                    nc.sync.dma_start(out=out_d[g * grp + j], in_=ob)
                return run

            for mf in range(FC):
                ops.append(a1_chunk(mf))
            for j in range(grp):
                ops.append(ff_j(j))
            return ops

        prev_d = make_d(g, h2T, o1s)

    for d in prev_d:
        d()


BIAS_NAMES = ("bq", "bk", "b1p", "bo2", "b2")


def build_nc(bl=BL, grp=GRP, use_bias=True):
    nc = bacc.Bacc("TRN2", target_bir_lowering=False, debug=False,
                   enable_asserts=True)
    io = {}

    def inp(name, shape, dt):
        io[name] = nc.dram_tensor(name, shape, dt, kind="ExternalInput").ap()

    inp("x", [bl, T, D], F32)
    inp("wq", [D, D], F16)
    inp("wk", [D, D], F16)
    inp("wv", [D, D], F16)
    inp("wo", [D, D], F16)
    inp("w1", [D, DFF], F16)
    inp("w2", [DFF, D], F16)
    if use_bias:
        inp("bq", [D], F32)
        inp("bk", [D], F32)
        inp("b1p", [DFF], F32)
        inp("bo2", [D], F32)
        inp("b2", [D], F32)
    io["out"] = nc.dram_tensor("out", [bl, T, D], F32,
                               kind="ExternalOutput").ap()

    with tile.TileContext(nc) as tc:
        build_decoder_block(tc, io, bl, grp, use_bias)
    nc.compile()
    return nc


def prep_weights(Wq, Wk, Wv, Wo, bo, W1, b1, W2, b2, g1, be1, g2, be2):
    f = np.float64
    Wq, Wk, Wv, Wo = (np.asarray(a, f) for a in (Wq, Wk, Wv, Wo))
    W1, W2 = np.asarray(W1, f), np.asarray(W2, f)
    g1, be1, g2, be2 = (np.asarray(a, f) for a in (g1, be1, g2, be2))
    bo, b1, b2 = np.asarray(bo, f), np.asarray(b1, f), np.asarray(b2, f)
    return {
        "wq": (g1[:, None] * Wq * SCALE).astype(np.float16),
        "wk": (g1[:, None] * Wk).astype(np.float16),
        "wv": (g1[:, None] * Wv).astype(np.float16),
        "wo": Wo.astype(np.float16),
        "w1": (g2[:, None] * W1).astype(np.float16),
        "w2": W2.astype(np.float16),
        "bq": (be1 @ Wq * SCALE).astype(np.float32),
        "bk": (be1 @ Wk).astype(np.float32),
        "b1p": (b1 + be2 @ W1).astype(np.float32),
        "bo2": (bo + (be1 @ Wv) @ Wo).astype(np.float32),
        "b2": b2.astype(np.float32),
    }


_NC_CACHE = {}


def get_nc(bl=BL, grp=GRP, use_bias=True):
    key = (bl, grp, use_bias)
    if key not in _NC_CACHE:
        _NC_CACHE[key] = build_nc(bl, grp, use_bias)
    return _NC_CACHE[key]


def prepare(inputs):
    """Build (or fetch) the Bass module and the per-core input maps for the
    given full inputs.  Shared by kernel() and external harnesses."""
    x = np.asarray(inputs["x"], np.float32)
    w = prep_weights(**{k: v for k, v in inputs.items() if k != "x"})
    use_bias = any(np.any(w[k]) for k in BIAS_NAMES)
    if not use_bias:
        w = {k: v for k, v in w.items() if k not in BIAS_NAMES}
    nc = get_nc(use_bias=use_bias)
    in_maps = []
    for c in range(N_CORES):
        m = dict(w)
        m["x"] = np.ascontiguousarray(x[c * BL:(c + 1) * BL])
        in_maps.append(m)
    return nc, in_maps


def kernel(**inputs):
    from concourse.bass_utils import run_bass_kernel_spmd

    nc, in_maps = prepare(inputs)
    res = run_bass_kernel_spmd(nc, in_maps, list(range(N_CORES)))
    return np.concatenate([r["out"] for r in res.results], axis=0)

